# revision 25
# baseline (speedup 1.0000x reference)
"""Multi-head self-attention Trainium2 Bass kernel (B=2, T=4096, D=512, H=8).

Sharding: 8 cores, each handles (batch b = core//4, head-pair hp = core%4).
Per core, for its 2 heads (host pre-transposes x and pre-scales Wq by 1/8,
all in bf16; the bk bias is dropped entirely — a per-q-column score offset
cancels exactly in softmax):
    kT = Wk @ x.T            ([128, T] bf16: head h on partitions 64h..+63)
    v  = x @ Wv.T            (bf16, interleaved with ones columns)
    qT = Wq' @ x.T + bq'     (q/k/v projections interleaved per x chunk)
    flash attention without max-subtraction (scores in ~[-9, 9], f32 exp ok):
      per 512-wide q block, per 128-wide kv tile:
        S.T_h0 / S.T_h1 computed as a CONCURRENT row-tiled matmul pair
            into one [128, 1024] PSUM tile (2 banks; ring of 3 tiles shared
            with the stage-D output matmuls)
        P.T = exp(S.T) -> bf16. Split between ScalarE (LUT exp ACTIVATE)
            and VectorE (Schraudolph: y = s*128/ln2 + (127-.0597)*128 as
            int16 -> bf16 bit pattern) to balance engine load; both engines
            are the kernel's critical resource (~1.2-1.3us per tile).
        ctxT_h[+l_h] += vaug_h.T @ P.T_h      (single [66, 2, 512] PSUM
                                               accumulator tile; rows 64-65=l)
    per-q-block tail: rl = 1/l via DVE reciprocal read straight out of PSUM,
    GpSimd partition_broadcast replicates rl to all 128 partitions, and the
    ctx extraction multiplies by rl while casting to bf16. Normalized ctx
    lets the output projection collapse to ONE full-K (128) matmul per
    128-q chunk (heads summed in PSUM), then copy+DMA. These run one
    q-block behind, inside the exp shadow, borrowing the st PSUM ring.
Host gathers: out[b] = sum of 4 cores' partials + (bv @ Wo.T + bo); the
v/o biases fold out exactly because softmax rows sum to 1. The last q block
ships unnormalized per-head partials (pa7/pb7) + l7; host divides.

This walrus build accepts at most ONE sync wait per instruction;
split_excess_waits() moves extras onto no-ops. walrus's LDW-dedup pass
(--enable-ldw-opt) rejects row-tiled LDWEIGHTS, so it stays disabled.
"""

import numpy as np
import ml_dtypes

import concourse.bass as bass
import concourse.tile as tile
from concourse import mybir
from concourse.bass_utils import run_bass_kernel_spmd

F32 = mybir.dt.float32
BF16 = mybir.dt.bfloat16
I16 = mybir.dt.int16

N_CORES = 8
B, T, D, H = 2, 4096, 512, 8
DK = D // H          # 64
TT = T // 128        # 32 kv tiles
KC = D // 128        # 4 contraction chunks
QB = 512             # q block width
NQB = T // QB        # 8 q blocks
VW = 132             # vaug cols per kv tile: [v_h0(64) one one v_h1(64) one one]

# Schraudolph exp in bf16 bit domain: bf16(bits(round(s*EXPA + EXPB))) ~ e^s
EXPA = 128.0 / float(np.log(2.0))
EXPB = (127.0 - 0.0597) * 128.0
N_DVE_EXP = 15            # kv tiles per q block expd on VectorE (of TT)

_split_ctr = [0]


def split_excess_waits(nc, limit=1):
    """walrus codegen in this toolchain accepts at most `limit` sync waits
    per instruction; move the excess onto nofuse NoOps inserted right before
    on the same engine (engines execute in order, semantics unchanged)."""
    n_split = 0
    for fn in nc.m.functions:
        blocks = fn.blocks if isinstance(fn.blocks, list) else list(fn.blocks.values())
        for blk in blocks:
            out = []
            for inst in blk.instructions:
                si = inst.sync_info
                if si is not None and len(si.on_wait) > limit:
                    waits = list(si.on_wait)
                    excess, keep = waits[:-limit], waits[-limit:]
                    for w in excess:
                        _split_ctr[0] += 1
                        out.append(mybir.InstNoOp(
                            name=f"I-wsplit-{_split_ctr[0]}",
                            opcode="NoOp",
                            engine=inst.engine,
                            sync_info=mybir.SyncInfo(on_wait=[w], on_update=[]),
                            bass_nofuse=True,
                        ))
                        n_split += 1
                    inst.sync_info = mybir.SyncInfo(
                        on_wait=keep, on_update=list(si.on_update))
                out.append(inst)
            blk.instructions[:] = out
    return n_split


def _ap(src, pattern):
    """Raw AP view over the same tensor/offset with an explicit
    [[stride, size], ...] element-stride pattern (partition dim first)."""
    return bass.AP(tensor=src.tensor, offset=src.offset,
                   ap=[list(d) for d in pattern])


def _dve_exp_tiles():
    """Evenly spread N_DVE_EXP of TT kv tiles onto the DVE exp path."""
    return {kb for kb in range(TT)
            if ((kb + 1) * N_DVE_EXP) // TT > (kb * N_DVE_EXP) // TT}


def build_kernel():
    nc = bass.Bass()
    xbT = nc.dram_tensor("xbT", [D, T], BF16, kind="ExternalInput")
    wqT = nc.dram_tensor("wqT", [D, 128], BF16, kind="ExternalInput")
    wkT = nc.dram_tensor("wkT", [D, 128], BF16, kind="ExternalInput")
    wvT = nc.dram_tensor("wvT", [D, 128], BF16, kind="ExternalInput")
    woT = nc.dram_tensor("woT", [128, D], F32, kind="ExternalInput")
    bq = nc.dram_tensor("bq", [128, 1], F32, kind="ExternalInput")
    part = nc.dram_tensor("part", [T, D], F32, kind="ExternalOutput")
    pa7 = nc.dram_tensor("pa7", [QB, D], F32, kind="ExternalOutput")
    pb7 = nc.dram_tensor("pb7", [QB, D], F32, kind="ExternalOutput")
    l7 = nc.dram_tensor("l7", [1, 2 * QB], F32, kind="ExternalOutput")

    dve_exp = _dve_exp_tiles()

    with tile.TileContext(nc) as tc:
        with tc.tile_pool(name="persist", bufs=1) as persist:
            # ---- persistent SBUF. Weights are loaded first (they gate the
            #      first projection matmuls); x streams per 512-col n-block
            #      in one 3D DMA each so the q-block-0 compute can start on
            #      block 0 without waiting for the whole 4MB load. ----
            wqt = persist.tile([128, KC, 128], BF16)
            nc.sync.dma_start(out=wqt, in_=wqT.rearrange("(c p) m -> p c m", p=128))
            wkt = persist.tile([128, KC, 128], BF16)
            nc.scalar.dma_start(out=wkt, in_=wkT.rearrange("(c p) m -> p c m", p=128))
            wvt = persist.tile([128, KC, 128], BF16)
            bq_t = persist.tile([128, 1], F32)
            nc.gpsimd.dma_start(out=bq_t, in_=bq[:, :])
            woTf = persist.tile([128, D], F32)
            nc.gpsimd.dma_start(out=woTf, in_=woT[:, :])
            woTs = persist.tile([128, D], BF16)
            nc.vector.tensor_copy(out=woTs, in_=woTf)  # noqa: cast to bf16

            xT = []
            for n in range(T // 512):
                xn = persist.tile([128, KC, 512], BF16, name=f"xT{n}")
                xT.append(xn)
                if n == 0:
                    # first block: split by chunk across both queues so the
                    # first projection matmul can start ASAP
                    for c in range(KC):
                        eng = nc.sync if (c % 2 == 0) else nc.scalar
                        eng.dma_start(
                            out=xn[:, c, :],
                            in_=xbT[128 * c: 128 * (c + 1), 0:512])
                    nc.sync.dma_start(
                        out=wvt,
                        in_=wvT.rearrange("(c p) m -> p c m", p=128))
                else:
                    eng = nc.sync if (n % 2 == 0) else nc.scalar
                    eng.dma_start(
                        out=xn,
                        in_=_ap(xbT[0:128, 512 * n: 512 * (n + 1)],
                                [[T, 128], [128 * T, KC], [1, 512]]))

            qT2 = persist.tile([128, T], BF16)   # heads stacked [h0|h1]
            kT2 = persist.tile([128, T], BF16)
            vaug = persist.tile([128, TT, VW], BF16)
            # ones columns (cols 64,65 and 130,131 of each VW block), set once
            nc.vector.memset(vaug[:, :, 64:66], 1.0)
            nc.vector.memset(vaug[:, :, 130:132], 1.0)

            # ---- fused projections + flash attention + output projection --
            with tc.tile_pool(name="stp", bufs=3, space="PSUM") as stp, \
                 tc.tile_pool(name="ctxp", bufs=1, space="PSUM") as ctxp, \
                 tc.tile_pool(name="ptp", bufs=6) as ptp, \
                 tc.tile_pool(name="drp", bufs=2, space="DRAM") as drp, \
                 tc.tile_pool(name="sC", bufs=2) as sC, \
                 tc.tile_pool(name="sD", bufs=3) as sD:

                saved = {}   # qb -> (ctx_sb bf16, rl_t or None-for-last)

                def stage_b_block(n):
                    """q/k/v projections for x block n, PSUM from the shared
                    st ring: one tile for q|k, one for the four v subtiles."""
                    sl = slice(512 * n, 512 * (n + 1))
                    ps_qk = stp.tile([128, 1024], F32, tag="st",
                                     name=f"psqk_{n}")
                    for c in range(KC):
                        nc.tensor.matmul(
                            ps_qk[:, 0:512], wqt[:, c, :], xT[n][:, c, :],
                            start=(c == 0), stop=(c == KC - 1))
                    for c in range(KC):
                        nc.tensor.matmul(
                            ps_qk[:, 512:1024], wkt[:, c, :], xT[n][:, c, :],
                            start=(c == 0), stop=(c == KC - 1))
                    nc.vector.tensor_scalar_add(
                        out=qT2[:, sl], in0=ps_qk[:, 0:512], scalar1=bq_t)
                    nc.scalar.activation(
                        out=kT2[:, sl], in_=ps_qk[:, 512:1024],
                        func=mybir.ActivationFunctionType.Copy)
                    ps_v4 = stp.tile([128, 1024], F32, tag="st",
                                     name=f"psv4_{n}")
                    for j in range(4):
                        for c in range(KC):
                            nc.tensor.matmul(
                                ps_v4[:, 128 * j: 128 * (j + 1)],
                                xT[n][:, c, 128 * j: 128 * (j + 1)],
                                wvt[:, c, :],
                                start=(c == 0), stop=(c == KC - 1))
                    # vaug[:, i, [0:64, 66:130]] <- v subtile pairs, strided
                    for half, eng_v in ((0, True), (1, False)):
                        i0 = 4 * n + 2 * half
                        base = vaug[:, i0, 0:64]
                        dst = bass.AP(
                            tensor=base.tensor, offset=base.offset,
                            ap=[list(base.ap[0]), [VW, 2], [66, 2], [1, 64]])
                        src = ps_v4[:, 256 * half: 256 * (half + 1)].rearrange(
                            "p (j two f) -> p j two f", j=2, two=2)
                        if eng_v:
                            nc.vector.tensor_copy(out=dst, in_=src)
                        else:
                            nc.scalar.activation(
                                out=dst, in_=src,
                                func=mybir.ActivationFunctionType.Copy)

                def stage_d_chunk(qb, c):
                    ctx_sb, rl_t = saved[qb]
                    ps_d = stp.tile([128, 1024], F32, tag="st",
                                    name=f"psd_{qb}_{c}")
                    nc.tensor.matmul(
                        ps_d[:, 0:512], ctx_sb[0:64, 128 * c: 128 * (c + 1)],
                        woTs[0:64, :], start=True, stop=True)
                    nc.tensor.matmul(
                        ps_d[:, 512:1024], ctx_sb[64:128, 128 * c: 128 * (c + 1)],
                        woTs[64:128, :], start=True, stop=True)
                    if rl_t is None:
                        # last q block: ship both head halves; host divides
                        oa = sD.tile([128, 512], F32, tag="od", name=f"oa7_{c}")
                        nc.vector.tensor_copy(out=oa, in_=ps_d[:, 0:512])
                        ob = sD.tile([128, 512], F32, tag="od", name=f"ob7_{c}")
                        nc.scalar.activation(
                            out=ob, in_=ps_d[:, 512:1024],
                            func=mybir.ActivationFunctionType.Copy)
                        nc.sync.dma_start(
                            out=pa7[128 * c: 128 * (c + 1), :], in_=oa)
                        nc.sync.dma_start(
                            out=pb7[128 * c: 128 * (c + 1), :], in_=ob)
                        return
                    tmp = sD.tile([128, 512], F32, tag="tmp",
                                  name=f"tmp_{qb}_{c}")
                    nc.scalar.activation(
                        out=tmp, in_=ps_d[:, 512:1024],
                        func=mybir.ActivationFunctionType.Identity,
                        scale=rl_t[:, 1, c:c + 1])
                    ost = sD.tile([128, 512], F32, tag="od",
                                  name=f"ost_{qb}_{c}")
                    nc.vector.scalar_tensor_tensor(
                        out=ost, in0=ps_d[:, 0:512], scalar=rl_t[:, 0, c:c + 1],
                        in1=tmp, op0=mybir.AluOpType.mult,
                        op1=mybir.AluOpType.add)
                    r0 = QB * qb + 128 * c
                    nc.sync.dma_start(out=part[r0: r0 + 128, :], in_=ost)

                def pv_pair(kb, pt, psc):
                    nc.tensor.matmul(
                        psc[:, 0, :], vaug[:, kb, 0:66],
                        pt[:, 0:512],
                        start=(kb == 0), stop=(kb == TT - 1))
                    nc.tensor.matmul(
                        psc[:, 1, :], vaug[:, kb, 66:132],
                        pt[:, 512:1024],
                        start=(kb == 0), stop=(kb == TT - 1))

                def tail(qb, psc):
                    """After the last PV of q block qb: pull l, start the
                    1/l transpose bounce, extract ctx to bf16."""
                    l_sb = sC.tile([1, 2, 512], F32, tag="lsb",
                                   name=f"lsb_{qb}")
                    nc.scalar.activation(
                        out=l_sb, in_=psc[64:65, :, :],
                        func=mybir.ActivationFunctionType.Copy)
                    ctx_sb = sC.tile([128, QB], BF16, tag="ctx",
                                     name=f"ctx_{qb}")
                    nc.vector.tensor_copy(out=ctx_sb[0:64, :],
                                          in_=psc[0:64, 0, :])
                    nc.vector.tensor_copy(out=ctx_sb[64:128, :],
                                          in_=psc[0:64, 1, :])
                    if qb == 0:
                        nc.gpsimd.dma_start(
                            out=l7[:, :],
                            in_=l_sb.rearrange("p two f -> p (two f)"))
                        saved[qb] = (ctx_sb, None)
                    else:
                        ld = drp.tile([1, 2 * QB], F32, tag="ld",
                                      name=f"ld_{qb}")
                        nc.gpsimd.dma_start(
                            out=ld, in_=l_sb.rearrange("p two f -> p (two f)"))
                        # transposed read-back: l_t[p, h, i] = l[512h+128i+p]
                        l_t = sC.tile([128, 2, 4], F32, tag="lt",
                                      name=f"lt_{qb}")
                        nc.gpsimd.dma_start(
                            out=l_t, in_=_ap(ld, [[1, 128], [QB, 2], [128, 4]]))
                        rl_t = sC.tile([128, 2, 4], F32, tag="rlt",
                                       name=f"rlt_{qb}")
                        nc.vector.reciprocal(rl_t, l_t)
                        saved[qb] = (ctx_sb, rl_t)

                # ---- flat software pipeline over (qb, kv-tile-pair).
                # kv tiles processed in PAIRS: both S matmul pairs
                # back-to-back, then both PV pairs — halves the costly
                # row-tiled <-> full-K PE reconfigurations per tile. The
                # PV queue stays >=4 tiles behind and flows ACROSS q-block
                # boundaries so the next block's S work hides the final
                # exp->PV drain of the previous block.
                stage_b_block(0)
                stage_b_block(1)
                pv_pending = []

                def pop_pv():
                    kb, pt, psc, iqb = pv_pending.pop(0)
                    pv_pair(kb, pt, psc)
                    if kb == TT - 1:
                        tail(iqb, psc)

                ps_c = None
                for qb in range(NQB):
                    qsl = slice(QB * qb, QB * (qb + 1))
                    for kb2 in range(0, TT, 2):
                        if kb2 == 0:
                            # ctx accumulator: [66, 2 heads, 512 q] = 2 banks
                            ps_c = ctxp.tile([66, 2, 512], F32, tag="ctx",
                                             name=f"psc_{qb}")
                        sts = []
                        for kb in (kb2, kb2 + 1):
                            st = stp.tile([128, 1024], F32, tag="st",
                                          name=f"st_{qb}_{kb}")
                            nc.tensor.matmul(
                                st[:, 0:512],
                                kT2[0:64, 128 * kb: 128 * (kb + 1)],
                                qT2[0:64, qsl], start=True, stop=True)
                            nc.tensor.matmul(
                                st[:, 512:1024],
                                kT2[64:128, 128 * kb: 128 * (kb + 1)],
                                qT2[64:128, qsl], start=True, stop=True)
                            sts.append((kb, st))
                        for kb, st in sts:
                            pt = ptp.tile([128, 1024], BF16, tag="pt",
                                          name=f"pt_{qb}_{kb}")
                            if kb in dve_exp:
                                nc.vector.tensor_scalar(
                                    out=pt.bitcast(I16), in0=st,
                                    scalar1=EXPA, scalar2=EXPB,
                                    op0=mybir.AluOpType.mult,
                                    op1=mybir.AluOpType.add)
                            else:
                                nc.scalar.activation(
                                    out=pt, in_=st,
                                    func=mybir.ActivationFunctionType.Exp)
                            pv_pending.append((kb, pt, ps_c, qb))
                        if qb == 0 and kb2 % 4 == 2 and kb2 // 4 + 2 < T // 512:
                            stage_b_block(kb2 // 4 + 2)
                        if qb > 0 and kb2 in (8, 14, 20, 26):
                            stage_d_chunk(qb - 1, (kb2 - 8) // 6)
                        while len(pv_pending) > 4:
                            pop_pv()
                while pv_pending:
                    pop_pv()
                for c in range(4):
                    stage_d_chunk(NQB - 1, c)

    split_excess_waits(nc)
    return nc


_NC_CACHE = None


def _get_nc():
    global _NC_CACHE
    if _NC_CACHE is None:
        _NC_CACHE = build_kernel()
    return _NC_CACHE


def make_in_maps(x, Wq, bq, Wk, bk, Wv, bv, Wo, bo):
    scale = 1.0 / np.sqrt(DK)
    bf = ml_dtypes.bfloat16
    in_maps = []
    for core in range(N_CORES):
        b, hp = divmod(core, 4)
        R = slice(128 * hp, 128 * hp + 128)
        in_maps.append({
            "xbT": np.ascontiguousarray(x[b].T.astype(bf)),
            "wqT": np.ascontiguousarray((Wq[R] * scale).T.astype(bf)),
            "wkT": np.ascontiguousarray(Wk[R].T.astype(bf)),
            "wvT": np.ascontiguousarray(Wv[R].T.astype(bf)),
            "woT": np.ascontiguousarray(Wo[:, R].T, dtype=np.float32),
            "bq": np.ascontiguousarray(
                (bq[R] * scale).reshape(128, 1), dtype=np.float32),
        })
    return in_maps


def kernel(x, Wq, bq, Wk, bk, Wv, bv, Wo, bo):
    x = np.asarray(x, dtype=np.float32)
    Wq, Wk, Wv, Wo = (np.asarray(a, dtype=np.float32) for a in (Wq, Wk, Wv, Wo))
    bq, bk, bv, bo = (np.asarray(a, dtype=np.float32) for a in (bq, bk, bv, bo))

    nc = _get_nc()
    in_maps = make_in_maps(x, Wq, bq, Wk, bk, Wv, bv, Wo, bo)
    res = run_bass_kernel_spmd(nc, in_maps, list(range(N_CORES)))
    parts = []
    for c in range(N_CORES):
        p = np.array(res.results[c]["part"])
        la = res.results[c]["l7"][0, 0:QB].astype(np.float64)
        lb = res.results[c]["l7"][0, QB:2 * QB].astype(np.float64)
        p[0:QB] = (res.results[c]["pa7"] / la[:, None] +
                   res.results[c]["pb7"] / lb[:, None])
        parts.append(p)

    # bk only shifts every score in a q column equally -> softmax-invariant,
    # so it is dropped on device. bv/bo contributions fold out exactly too.
    bcorr = (bv @ Wo.T + bo).astype(np.float32)
    out = np.empty((B, T, D), dtype=np.float32)
    for b in range(B):
        acc = parts[4 * b].astype(np.float64)
        for c in range(4 * b + 1, 4 * b + 4):
            acc += parts[c]
        out[b] = (acc + bcorr).astype(np.float32)
    return out


# revision 26
# speedup vs baseline: 1.0159x; 1.0159x over previous
"""Multi-head self-attention Trainium2 Bass kernel (B=2, T=4096, D=512, H=8).

Sharding: 8 cores, each handles (batch b = core//4, head-pair hp = core%4).
Per core, for its 2 heads (host pre-transposes x and pre-scales Wq by 1/8,
all in bf16; the bk bias is dropped entirely — a per-q-column score offset
cancels exactly in softmax):
    kT = Wk @ x.T            ([128, T] bf16: head h on partitions 64h..+63)
    v  = x @ Wv.T            (bf16, interleaved with ones columns)
    qT = Wq' @ x.T + bq'     (q/k/v projections interleaved per x chunk)
    flash attention without max-subtraction (scores in ~[-9, 9], f32 exp ok):
      per 512-wide q block, per 128-wide kv tile:
        S.T_h0 / S.T_h1 computed as a CONCURRENT row-tiled matmul pair
            into one [128, 1024] PSUM tile (2 banks; ring of 3 tiles shared
            with the stage-D output matmuls)
        P.T = exp(S.T) -> bf16. Split between ScalarE (LUT exp ACTIVATE)
            and VectorE (Schraudolph: y = s*128/ln2 + (127-.0597)*128 as
            int16 -> bf16 bit pattern) to balance engine load; both engines
            are the kernel's critical resource (~1.2-1.3us per tile).
        ctxT_h[+l_h] += vaug_h.T @ P.T_h      (single [66, 2, 512] PSUM
                                               accumulator tile; rows 64-65=l)
    per-q-block tail: rl = 1/l via DVE reciprocal read straight out of PSUM,
    GpSimd partition_broadcast replicates rl to all 128 partitions, and the
    ctx extraction multiplies by rl while casting to bf16. Normalized ctx
    lets the output projection collapse to ONE full-K (128) matmul per
    128-q chunk (heads summed in PSUM), then copy+DMA. These run one
    q-block behind, inside the exp shadow, borrowing the st PSUM ring.
Host gathers: out[b] = sum of 4 cores' partials + (bv @ Wo.T + bo); the
v/o biases fold out exactly because softmax rows sum to 1. The last q block
ships unnormalized per-head partials (pa7/pb7) + l7; host divides.

This walrus build accepts at most ONE sync wait per instruction;
split_excess_waits() moves extras onto no-ops. walrus's LDW-dedup pass
(--enable-ldw-opt) rejects row-tiled LDWEIGHTS, so it stays disabled.
"""

import numpy as np
import ml_dtypes

import concourse.bass as bass
import concourse.tile as tile
from concourse import mybir
from concourse.bass_utils import run_bass_kernel_spmd

F32 = mybir.dt.float32
BF16 = mybir.dt.bfloat16
I16 = mybir.dt.int16

N_CORES = 8
B, T, D, H = 2, 4096, 512, 8
DK = D // H          # 64
TT = T // 128        # 32 kv tiles
KC = D // 128        # 4 contraction chunks
QB = 512             # q block width
NQB = T // QB        # 8 q blocks
VW = 132             # vaug cols per kv tile: [v_h0(64) one one v_h1(64) one one]

# Schraudolph exp in bf16 bit domain: bf16(bits(round(s*EXPA + EXPB))) ~ e^s
EXPA = 128.0 / float(np.log(2.0))
EXPB = (127.0 - 0.0597) * 128.0
N_DVE_EXP = 15            # kv tiles per q block expd on VectorE (of TT)

_split_ctr = [0]


def split_excess_waits(nc, limit=1):
    """walrus codegen in this toolchain accepts at most `limit` sync waits
    per instruction; move the excess onto nofuse NoOps inserted right before
    on the same engine (engines execute in order, semantics unchanged)."""
    n_split = 0
    for fn in nc.m.functions:
        blocks = fn.blocks if isinstance(fn.blocks, list) else list(fn.blocks.values())
        for blk in blocks:
            out = []
            for inst in blk.instructions:
                si = inst.sync_info
                if si is not None and len(si.on_wait) > limit:
                    waits = list(si.on_wait)
                    excess, keep = waits[:-limit], waits[-limit:]
                    for w in excess:
                        _split_ctr[0] += 1
                        out.append(mybir.InstNoOp(
                            name=f"I-wsplit-{_split_ctr[0]}",
                            opcode="NoOp",
                            engine=inst.engine,
                            sync_info=mybir.SyncInfo(on_wait=[w], on_update=[]),
                            bass_nofuse=True,
                        ))
                        n_split += 1
                    inst.sync_info = mybir.SyncInfo(
                        on_wait=keep, on_update=list(si.on_update))
                out.append(inst)
            blk.instructions[:] = out
    return n_split


def _ap(src, pattern):
    """Raw AP view over the same tensor/offset with an explicit
    [[stride, size], ...] element-stride pattern (partition dim first)."""
    return bass.AP(tensor=src.tensor, offset=src.offset,
                   ap=[list(d) for d in pattern])


def _dve_exp_tiles():
    """Evenly spread N_DVE_EXP of TT kv tiles onto the DVE exp path."""
    return {kb for kb in range(TT)
            if ((kb + 1) * N_DVE_EXP) // TT > (kb * N_DVE_EXP) // TT}


def build_kernel():
    nc = bass.Bass()
    xbT = nc.dram_tensor("xbT", [D, T], BF16, kind="ExternalInput")
    wqT = nc.dram_tensor("wqT", [D, 128], BF16, kind="ExternalInput")
    wkT = nc.dram_tensor("wkT", [D, 128], BF16, kind="ExternalInput")
    wvT = nc.dram_tensor("wvT", [D, 128], BF16, kind="ExternalInput")
    woT = nc.dram_tensor("woT", [128, D], F32, kind="ExternalInput")
    bq = nc.dram_tensor("bq", [128, 1], F32, kind="ExternalInput")
    part = nc.dram_tensor("part", [T, D], F32, kind="ExternalOutput")
    pa7 = nc.dram_tensor("pa7", [QB, D], F32, kind="ExternalOutput")
    pb7 = nc.dram_tensor("pb7", [QB, D], F32, kind="ExternalOutput")
    l7 = nc.dram_tensor("l7", [1, 2 * QB], F32, kind="ExternalOutput")

    dve_exp = _dve_exp_tiles()

    with tile.TileContext(nc) as tc:
        with tc.tile_pool(name="persist", bufs=1) as persist:
            # ---- persistent SBUF. Weights are loaded first (they gate the
            #      first projection matmuls); x streams per 512-col n-block
            #      in one 3D DMA each so the q-block-0 compute can start on
            #      block 0 without waiting for the whole 4MB load. ----
            wqt = persist.tile([128, KC, 128], BF16)
            nc.sync.dma_start(out=wqt, in_=wqT.rearrange("(c p) m -> p c m", p=128))
            wkt = persist.tile([128, KC, 128], BF16)
            nc.scalar.dma_start(out=wkt, in_=wkT.rearrange("(c p) m -> p c m", p=128))
            wvt = persist.tile([128, KC, 128], BF16)
            bq_t = persist.tile([128, 1], F32)
            nc.gpsimd.dma_start(out=bq_t, in_=bq[:, :])
            woTf = persist.tile([128, D], F32)
            nc.gpsimd.dma_start(out=woTf, in_=woT[:, :])
            woTs = persist.tile([128, D], BF16)
            nc.vector.tensor_copy(out=woTs, in_=woTf)  # noqa: cast to bf16

            xT = []
            for n in range(T // 512):
                xn = persist.tile([128, KC, 512], BF16, name=f"xT{n}")
                xT.append(xn)
                if n == 0:
                    # first block: split by chunk across both queues so the
                    # first projection matmul can start ASAP
                    for c in range(KC):
                        eng = nc.sync if (c % 2 == 0) else nc.scalar
                        eng.dma_start(
                            out=xn[:, c, :],
                            in_=xbT[128 * c: 128 * (c + 1), 0:512])
                    nc.sync.dma_start(
                        out=wvt,
                        in_=wvT.rearrange("(c p) m -> p c m", p=128))
                else:
                    eng = nc.sync if (n % 2 == 0) else nc.scalar
                    eng.dma_start(
                        out=xn,
                        in_=_ap(xbT[0:128, 512 * n: 512 * (n + 1)],
                                [[T, 128], [128 * T, KC], [1, 512]]))

            qT2 = persist.tile([128, T], BF16)   # heads stacked [h0|h1]
            kT2 = persist.tile([128, T], BF16)
            vaug = persist.tile([128, TT, VW], BF16)
            # ones columns (cols 64,65 and 130,131 of each VW block), set once
            nc.vector.memset(vaug[:, :, 64:66], 1.0)
            nc.vector.memset(vaug[:, :, 130:132], 1.0)

            # ---- fused projections + flash attention + output projection --
            with tc.tile_pool(name="stp", bufs=3, space="PSUM") as stp, \
                 tc.tile_pool(name="ctxp", bufs=1, space="PSUM") as ctxp, \
                 tc.tile_pool(name="ptp", bufs=6) as ptp, \
                 tc.tile_pool(name="drp", bufs=2, space="DRAM") as drp, \
                 tc.tile_pool(name="sC", bufs=2) as sC, \
                 tc.tile_pool(name="sD", bufs=3) as sD:

                saved = {}   # qb -> (ctx_sb bf16, rl_t or None-for-last)

                def stage_b_block(n):
                    """q/k/v projections for x block n, PSUM from the shared
                    st ring: one tile for q|k, one for the four v subtiles."""
                    sl = slice(512 * n, 512 * (n + 1))
                    ps_qk = stp.tile([128, 1024], F32, tag="st",
                                     name=f"psqk_{n}")
                    for c in range(KC):
                        nc.tensor.matmul(
                            ps_qk[:, 0:512], wqt[:, c, :], xT[n][:, c, :],
                            start=(c == 0), stop=(c == KC - 1))
                    for c in range(KC):
                        nc.tensor.matmul(
                            ps_qk[:, 512:1024], wkt[:, c, :], xT[n][:, c, :],
                            start=(c == 0), stop=(c == KC - 1))
                    nc.vector.tensor_scalar_add(
                        out=qT2[:, sl], in0=ps_qk[:, 0:512], scalar1=bq_t)
                    nc.scalar.activation(
                        out=kT2[:, sl], in_=ps_qk[:, 512:1024],
                        func=mybir.ActivationFunctionType.Copy)
                    ps_v4 = stp.tile([128, 1024], F32, tag="st",
                                     name=f"psv4_{n}")
                    for j in range(4):
                        for c in range(KC):
                            nc.tensor.matmul(
                                ps_v4[:, 128 * j: 128 * (j + 1)],
                                xT[n][:, c, 128 * j: 128 * (j + 1)],
                                wvt[:, c, :],
                                start=(c == 0), stop=(c == KC - 1))
                    # vaug[:, i, [0:64, 66:130]] <- v subtile pairs, strided
                    for half, eng_v in ((0, True), (1, False)):
                        i0 = 4 * n + 2 * half
                        base = vaug[:, i0, 0:64]
                        dst = bass.AP(
                            tensor=base.tensor, offset=base.offset,
                            ap=[list(base.ap[0]), [VW, 2], [66, 2], [1, 64]])
                        src = ps_v4[:, 256 * half: 256 * (half + 1)].rearrange(
                            "p (j two f) -> p j two f", j=2, two=2)
                        if eng_v:
                            nc.vector.tensor_copy(out=dst, in_=src)
                        else:
                            nc.scalar.activation(
                                out=dst, in_=src,
                                func=mybir.ActivationFunctionType.Copy)

                def stage_d_chunk(qb, c):
                    ctx_sb, rl_t = saved[qb]
                    ps_d = stp.tile([128, 1024], F32, tag="st",
                                    name=f"psd_{qb}_{c}")
                    nc.tensor.matmul(
                        ps_d[:, 0:512], ctx_sb[0:64, 128 * c: 128 * (c + 1)],
                        woTs[0:64, :], start=True, stop=True)
                    nc.tensor.matmul(
                        ps_d[:, 512:1024], ctx_sb[64:128, 128 * c: 128 * (c + 1)],
                        woTs[64:128, :], start=True, stop=True)
                    if rl_t is None:
                        # last q block: ship both head halves; host divides
                        oa = sD.tile([128, 512], F32, tag="od", name=f"oa7_{c}")
                        nc.vector.tensor_copy(out=oa, in_=ps_d[:, 0:512])
                        ob = sD.tile([128, 512], F32, tag="od", name=f"ob7_{c}")
                        nc.scalar.activation(
                            out=ob, in_=ps_d[:, 512:1024],
                            func=mybir.ActivationFunctionType.Copy)
                        nc.sync.dma_start(
                            out=pa7[128 * c: 128 * (c + 1), :], in_=oa)
                        nc.sync.dma_start(
                            out=pb7[128 * c: 128 * (c + 1), :], in_=ob)
                        return
                    tmp = sD.tile([128, 512], F32, tag="tmp",
                                  name=f"tmp_{qb}_{c}")
                    nc.scalar.activation(
                        out=tmp, in_=ps_d[:, 512:1024],
                        func=mybir.ActivationFunctionType.Identity,
                        scale=rl_t[:, 1, c:c + 1])
                    ost = sD.tile([128, 512], F32, tag="od",
                                  name=f"ost_{qb}_{c}")
                    nc.vector.scalar_tensor_tensor(
                        out=ost, in0=ps_d[:, 0:512], scalar=rl_t[:, 0, c:c + 1],
                        in1=tmp, op0=mybir.AluOpType.mult,
                        op1=mybir.AluOpType.add)
                    r0 = QB * qb + 128 * c
                    nc.sync.dma_start(out=part[r0: r0 + 128, :], in_=ost)

                def pv_pair(kb, pt, psc):
                    nc.tensor.matmul(
                        psc[:, 0, :], vaug[:, kb, 0:66],
                        pt[:, 0:512],
                        start=(kb == 0), stop=(kb == TT - 1))
                    nc.tensor.matmul(
                        psc[:, 1, :], vaug[:, kb, 66:132],
                        pt[:, 512:1024],
                        start=(kb == 0), stop=(kb == TT - 1))

                def tail(qb, psc):
                    """After the last PV of q block qb: pull l, start the
                    1/l transpose bounce, extract ctx to bf16."""
                    l_sb = sC.tile([1, 2, 512], F32, tag="lsb",
                                   name=f"lsb_{qb}")
                    nc.scalar.activation(
                        out=l_sb, in_=psc[64:65, :, :],
                        func=mybir.ActivationFunctionType.Copy)
                    ctx_sb = sC.tile([128, QB], BF16, tag="ctx",
                                     name=f"ctx_{qb}")
                    nc.vector.tensor_copy(out=ctx_sb[0:64, :],
                                          in_=psc[0:64, 0, :])
                    nc.vector.tensor_copy(out=ctx_sb[64:128, :],
                                          in_=psc[0:64, 1, :])
                    if qb == 0:
                        nc.gpsimd.dma_start(
                            out=l7[:, :],
                            in_=l_sb.rearrange("p two f -> p (two f)"))
                        saved[qb] = (ctx_sb, None)
                    else:
                        ld = drp.tile([1, 2 * QB], F32, tag="ld",
                                      name=f"ld_{qb}")
                        nc.gpsimd.dma_start(
                            out=ld, in_=l_sb.rearrange("p two f -> p (two f)"))
                        # transposed read-back: l_t[p, h, i] = l[512h+128i+p]
                        l_t = sC.tile([128, 2, 4], F32, tag="lt",
                                      name=f"lt_{qb}")
                        nc.gpsimd.dma_start(
                            out=l_t, in_=_ap(ld, [[1, 128], [QB, 2], [128, 4]]))
                        rl_t = sC.tile([128, 2, 4], F32, tag="rlt",
                                       name=f"rlt_{qb}")
                        nc.vector.reciprocal(rl_t, l_t)
                        saved[qb] = (ctx_sb, rl_t)

                # ---- flat software pipeline over (qb, kv-tile-pair).
                # kv tiles processed in PAIRS: both S matmul pairs
                # back-to-back, then both PV pairs — halves the costly
                # row-tiled <-> full-K PE reconfigurations per tile. The
                # PV queue stays >=4 tiles behind and flows ACROSS q-block
                # boundaries so the next block's S work hides the final
                # exp->PV drain of the previous block.
                stage_b_block(0)
                stage_b_block(1)
                pv_pending = []

                def pop_pv():
                    kb, pt, psc, iqb = pv_pending.pop(0)
                    pv_pair(kb, pt, psc)
                    if kb == TT - 1:
                        tail(iqb, psc)

                ps_c = None
                for qb in range(NQB):
                    qsl = slice(QB * qb, QB * (qb + 1))
                    for kb2 in range(0, TT, 2):
                        if kb2 == 0:
                            # ctx accumulator: [66, 2 heads, 512 q] = 2 banks
                            ps_c = ctxp.tile([66, 2, 512], F32, tag="ctx",
                                             name=f"psc_{qb}")
                        sts = []
                        for kb in (kb2, kb2 + 1):
                            st = stp.tile([128, 1024], F32, tag="st",
                                          name=f"st_{qb}_{kb}")
                            nc.tensor.matmul(
                                st[:, 0:512],
                                kT2[0:64, 128 * kb: 128 * (kb + 1)],
                                qT2[0:64, qsl], start=True, stop=True)
                            nc.tensor.matmul(
                                st[:, 512:1024],
                                kT2[64:128, 128 * kb: 128 * (kb + 1)],
                                qT2[64:128, qsl], start=True, stop=True)
                            sts.append((kb, st))
                        for kb, st in sts:
                            pt = ptp.tile([128, 1024], BF16, tag="pt",
                                          name=f"pt_{qb}_{kb}")
                            if kb >= TT - 2:
                                # last tiles gate the q-block drain: split
                                # each across BOTH engines to halve latency
                                nc.scalar.activation(
                                    out=pt[:, 0:512], in_=st[:, 0:512],
                                    func=mybir.ActivationFunctionType.Exp)
                                nc.vector.tensor_scalar(
                                    out=pt[:, 512:1024].bitcast(I16),
                                    in0=st[:, 512:1024],
                                    scalar1=EXPA, scalar2=EXPB,
                                    op0=mybir.AluOpType.mult,
                                    op1=mybir.AluOpType.add)
                            elif kb in dve_exp:
                                nc.vector.tensor_scalar(
                                    out=pt.bitcast(I16), in0=st,
                                    scalar1=EXPA, scalar2=EXPB,
                                    op0=mybir.AluOpType.mult,
                                    op1=mybir.AluOpType.add)
                            else:
                                nc.scalar.activation(
                                    out=pt, in_=st,
                                    func=mybir.ActivationFunctionType.Exp)
                            pv_pending.append((kb, pt, ps_c, qb))
                        if qb == 0 and kb2 % 4 == 2 and kb2 // 4 + 2 < T // 512:
                            stage_b_block(kb2 // 4 + 2)
                        if qb > 0 and kb2 in (8, 14, 20, 26):
                            stage_d_chunk(qb - 1, (kb2 - 8) // 6)
                        while len(pv_pending) > 4:
                            pop_pv()
                while pv_pending:
                    pop_pv()
                for c in range(4):
                    stage_d_chunk(NQB - 1, c)

    split_excess_waits(nc)
    return nc


_NC_CACHE = None


def _get_nc():
    global _NC_CACHE
    if _NC_CACHE is None:
        _NC_CACHE = build_kernel()
    return _NC_CACHE


def make_in_maps(x, Wq, bq, Wk, bk, Wv, bv, Wo, bo):
    scale = 1.0 / np.sqrt(DK)
    bf = ml_dtypes.bfloat16
    in_maps = []
    for core in range(N_CORES):
        b, hp = divmod(core, 4)
        R = slice(128 * hp, 128 * hp + 128)
        in_maps.append({
            "xbT": np.ascontiguousarray(x[b].T.astype(bf)),
            "wqT": np.ascontiguousarray((Wq[R] * scale).T.astype(bf)),
            "wkT": np.ascontiguousarray(Wk[R].T.astype(bf)),
            "wvT": np.ascontiguousarray(Wv[R].T.astype(bf)),
            "woT": np.ascontiguousarray(Wo[:, R].T, dtype=np.float32),
            "bq": np.ascontiguousarray(
                (bq[R] * scale).reshape(128, 1), dtype=np.float32),
        })
    return in_maps


def kernel(x, Wq, bq, Wk, bk, Wv, bv, Wo, bo):
    x = np.asarray(x, dtype=np.float32)
    Wq, Wk, Wv, Wo = (np.asarray(a, dtype=np.float32) for a in (Wq, Wk, Wv, Wo))
    bq, bk, bv, bo = (np.asarray(a, dtype=np.float32) for a in (bq, bk, bv, bo))

    nc = _get_nc()
    in_maps = make_in_maps(x, Wq, bq, Wk, bk, Wv, bv, Wo, bo)
    res = run_bass_kernel_spmd(nc, in_maps, list(range(N_CORES)))
    parts = []
    for c in range(N_CORES):
        p = np.array(res.results[c]["part"])
        la = res.results[c]["l7"][0, 0:QB].astype(np.float64)
        lb = res.results[c]["l7"][0, QB:2 * QB].astype(np.float64)
        p[0:QB] = (res.results[c]["pa7"] / la[:, None] +
                   res.results[c]["pb7"] / lb[:, None])
        parts.append(p)

    # bk only shifts every score in a q column equally -> softmax-invariant,
    # so it is dropped on device. bv/bo contributions fold out exactly too.
    bcorr = (bv @ Wo.T + bo).astype(np.float32)
    out = np.empty((B, T, D), dtype=np.float32)
    for b in range(B):
        acc = parts[4 * b].astype(np.float64)
        for c in range(4 * b + 1, 4 * b + 4):
            acc += parts[c]
        out[b] = (acc + bcorr).astype(np.float32)
    return out
